# revision 1
# baseline (speedup 1.0000x reference)
"""Trainium2 Bass kernel for nn_AlignmentAttention_82755429860169.

Mathematical collapse exploited: the reference broadcasts Q over the batch dim
(Qr rows are identical), so f_Q has identical rows, diff[i,j,:] is independent
of j, the pairwise [N,N] tensor is rank-deficient:
  - forward softmax over a constant row is exactly uniform (1/N)
  - backward softmax columns are identical
  - the navigator MLP only needs the 512 distinct rows, not 512*512
So:  loss = RHO*mean(c) + (1-RHO)*sum(c * softmax(d))   with
  c[i] = ||f_K[i] - q||^2,  d[i] = -(nav(mse_i)),  q = critic(Q).

Device work = critic over K ([512,256,512]) sharded 64 rows/core over 8 cores.
Layout: transposed ([e on partitions, s on free]); X.T tiles are produced by a
host-side transpose of each K row during sharding. The two big matmuls run in
fp8e4m3 with MatmulPerfMode.DoubleRow (2x PE throughput; measured end-to-end
rel err 2.3e-4 vs the fp32 reference); accumulation is fp32 (PSUM, and the
fused scalar_tensor_tensor accum_out row-sums on DVE). X is dual-loaded: fp8
(DR-interleaved) for the PE, bf16 for the DVE gate reduction, which uses
  I = sum_s tau*H' - sum_s (tau-1)*X   (two STT ops per row-chunk, no extra
elementwise passes; sigmoid/relu+bias are single fused ACT passes from PSUM).
Both X copies are host-packed so each row-pair loads with ONE contiguous DMA
per dtype (the DMA-issue path on the sync sequencer was the hidden bottleneck:
301 -> 103 descriptors), and the two X streams ride independent DMA queue
families (bf16 on sync/HWDGE, fp8 on gpsimd/SWDGE) - measured -19% vs sharing
one queue. c and d are tiny per-core vectors; the final
softmax-weighted scalar is assembled on the host from the 8 cores' outputs.

Measured on 8 axon-tunneled trn2 cores: ~114 us HW exec (within-session A/B:
113.7 vs 140.9 us for single-queue), ~114-160 us across sessions (reps=21 vs reps=1 programs, device-
resident inputs, async-batched executions; within-session A/B sweep picked
this config over merged-ACT and deeper-buffer variants). The cost-model
TimelineSim predicts 209 us with DVE as the binding engine (171 us busy);
hardware runs the DVE/ACT ladder faster than the model's errata-derated rates.
"""

import numpy as np
import ml_dtypes

import concourse.bass as bass
import concourse.mybir as mybir
import concourse.tile as tile
from concourse import bacc
from concourse.bass_utils import run_bass_kernel_spmd

BF = ml_dtypes.bfloat16
NP8 = ml_dtypes.float8_e4m3
F32 = mybir.dt.float32
BF16 = mybir.dt.bfloat16
F8 = mybir.dt.float8e4
AF = mybir.ActivationFunctionType
ALU = mybir.AluOpType

N_CORES = 8
N = 512
S = 256
E = 512
DIM = 256
HID = 512
ROWS = N // N_CORES        # 64 K-rows per core
EC = E // 128              # 4 e-chunks
DC = DIM // 128            # 2
HC = HID // 128            # 4
RHO = 0.5
NEG_SLOPE = 0.01


def _build(rows=ROWS, reps=1, mode="fp8dr", stt_split=False, deep_bufs=False,
           const_bias=None, x_bufs=3, dma_split=False):
    """reps>1 repeats the whole (idempotent) body for slope-based HW timing.

    mode="bf16": both big matmuls in bf16.
    mode="fp8dr": big matmuls in fp8e4m3 with DoubleRow (2x PE throughput);
      X is dual-loaded (fp8 for the PE, bf16 for the DVE gate reduction).
    stt_split: run the (tau-1)*X STT on GPSIMD instead of DVE.
    """
    nc = bacc.Bacc("TRN2", target_bir_lowering=False, debug=False, num_devices=N_CORES)

    npairs = rows // 2
    # packed X: per pair one contiguous [128, EC*2*S] block per dtype
    ktp = nc.dram_tensor("KTP", [npairs, 128, EC * 2 * S], BF16, kind="ExternalInput")
    qtp = nc.dram_tensor("QTP", [128, EC * S], BF16, kind="ExternalInput")
    if mode == "fp8dr":
        ktdp = nc.dram_tensor("KTDP", [npairs, 128, EC * 2 * S], F8, kind="ExternalInput")
        qtdp = nc.dram_tensor("QTDP", [128, EC * S], F8, kind="ExternalInput")
        wttd = nc.dram_tensor("WTTD", [128, EC * E], F8, kind="ExternalInput")
        whtd = nc.dram_tensor("WHTD", [128, EC * E], F8, kind="ExternalInput")
    else:
        wtt = nc.dram_tensor("WTT", [E, E], BF16, kind="ExternalInput")
        wht = nc.dram_tensor("WHT", [E, E], BF16, kind="ExternalInput")
    wc1t = nc.dram_tensor("WC1T", [E, DIM], BF16, kind="ExternalInput")
    wc2t = nc.dram_tensor("WC2T", [DIM, DIM], BF16, kind="ExternalInput")
    wn1t = nc.dram_tensor("WN1T", [DIM, HID], BF16, kind="ExternalInput")
    wn2t = nc.dram_tensor("WN2T", [HID, DIM], BF16, kind="ExternalInput")
    wn3t = nc.dram_tensor("WN3T", [DIM, 1], BF16, kind="ExternalInput")
    bt_d = nc.dram_tensor("BT", [E, 1], F32, kind="ExternalInput")
    bh_d = nc.dram_tensor("BH", [E, 1], F32, kind="ExternalInput")
    bc1_d = nc.dram_tensor("BC1", [DIM, 1], F32, kind="ExternalInput")
    bc2_d = nc.dram_tensor("BC2", [DIM, 1], F32, kind="ExternalInput")
    bn1_d = nc.dram_tensor("BN1", [HID, 1], F32, kind="ExternalInput")
    bn2_d = nc.dram_tensor("BN2", [DIM, 1], F32, kind="ExternalInput")
    bn3_d = nc.dram_tensor("BN3", [1, 1], F32, kind="ExternalInput")

    c_out = nc.dram_tensor("C_OUT", [1, rows], F32, kind="ExternalOutput")
    d_out = nc.dram_tensor("D_OUT", [1, rows], F32, kind="ExternalOutput")

    pairs = rows // 2
    NCOLS = rows + 1           # K rows + the shared Q row

    with tile.TileContext(nc) as tc:
        with tc.tile_pool(name="const", bufs=1) as cst, \
             tc.tile_pool(name="work", bufs=1) as work:

            for _rep in range(reps):
                # ---- stage 0: weights + biases to SBUF -------------------------
                def load_w(dram, ncol, tag):
                    tiles = []
                    nchunk = dram.shape[0] // 128
                    for k in range(nchunk):
                        t = cst.tile([128, ncol], BF16, tag=f"{tag}{k}")
                        nc.sync.dma_start(t[:], dram[k * 128:(k + 1) * 128, :])
                        tiles.append(t)
                    return tiles

                if mode == "fp8dr":
                    wttd_sb = cst.tile([128, EC * E], F8, tag="wttd")
                    whtd_sb = cst.tile([128, EC * E], F8, tag="whtd")
                    nc.sync.dma_start(wttd_sb[:], wttd[:, :])
                    nc.sync.dma_start(whtd_sb[:], whtd[:, :])
                    wtt3 = wttd_sb[:].rearrange("p (j o) -> p j o", j=EC)
                    wht3 = whtd_sb[:].rearrange("p (j o) -> p j o", j=EC)
                else:
                    wtt_sb = load_w(wtt, E, "wtt")
                    wht_sb = load_w(wht, E, "wht")
                wc1_sb = load_w(wc1t, DIM, "wc1")
                wc2_sb = load_w(wc2t, DIM, "wc2")
                wn1_sb = load_w(wn1t, HID, "wn1")
                wn2_sb = load_w(wn2t, DIM, "wn2")
                wn3_sb = load_w(wn3t, 1, "wn3")

                def load_b(dram, tag):
                    tiles = []
                    nchunk = dram.shape[0] // 128
                    for k in range(nchunk):
                        t = cst.tile([128, 1], F32, tag=f"{tag}{k}")
                        nc.sync.dma_start(t[:], dram[k * 128:(k + 1) * 128, :])
                        tiles.append(t)
                    return tiles

                bt_sb = load_b(bt_d, "bt")
                bh_sb = load_b(bh_d, "bh")
                bc1_sb = load_b(bc1_d, "bc1")
                bc2_sb = load_b(bc2_d, "bc2")
                bn1_sb = load_b(bn1_d, "bn1")
                bn2_sb = load_b(bn2_d, "bn2")
                nbn3_sb = cst.tile([128, 1], F32, tag="nbn3")
                bn3_sb = cst.tile([128, 1], F32, tag="bn3")
                nc.sync.dma_start(bn3_sb[0:1, :], bn3_d[:, :])
                nc.vector.tensor_scalar(out=nbn3_sb[0:1, :], in0=bn3_sb[0:1, :],
                                        scalar1=-1.0, scalar2=None, op0=ALU.mult)

                ones16 = cst.tile([128, 1], BF16, tag="ones")
                nc.vector.memset(ones16[:], 1.0)
                if const_bias is not None:
                    cbt = cst.tile([128, 1], F32, tag="cbt")
                    cbh = cst.tile([128, 1], F32, tag="cbh")
                    nc.vector.memset(cbt[:], float(const_bias[0]))
                    nc.vector.memset(cbh[:], float(const_bias[1]))

                # I accumulators (per e-chunk), col j = row j, col `rows` = Q
                acc_a = [cst.tile([128, NCOLS], F32, tag=f"acca{k}", name=f"acca{k}")
                         for k in range(EC)]
                i_sb = [cst.tile([128, NCOLS], F32, tag=f"isb{k}", name=f"isb{k}")
                        for k in range(EC)]

                # ---- stage 1: critic main loop --------------------------------
                def mm_into(ps_slice, o, xtiles, xdr, w3, w_sb, ncols_s):
                    if mode == "fp8dr":
                        for c in range(EC // 2):
                            nc.tensor.matmul(
                                ps_slice,
                                w3[:, 2 * c:2 * c + 2, o * 128:(o + 1) * 128],
                                xdr[c][:, :, 0:ncols_s],
                                start=(c == 0), stop=(c == EC // 2 - 1),
                                perf_mode=mybir.MatmulPerfMode.DoubleRow)
                    else:
                        for k in range(EC):
                            nc.tensor.matmul(ps_slice, w_sb[k][:, o * 128:(o + 1) * 128],
                                             xtiles[k][:, 0:ncols_s],
                                             start=(k == 0), stop=(k == EC - 1))

                def do_pair_merged(ps1, xtiles, ncols_s, col0, nrows_here, xdr):
                    # const-bias fast path: psum [128,1024] spans two o-chunks,
                    # one ACT instruction per weight per chunk-group.
                    for og in range(EC // 2):
                        ps_t = ps1.tile([128, 1024], F32, tag="psTm", bufs=2, name="ps_t")
                        ps_h = ps1.tile([128, 1024], F32, tag="psHm", bufs=2, name="ps_h")
                        for j in range(2):
                            o = 2 * og + j
                            w3t = wtt3 if mode == "fp8dr" else None
                            w3h = wht3 if mode == "fp8dr" else None
                            mm_into(ps_t[:, j * 512:j * 512 + ncols_s], o, xtiles, xdr,
                                    w3t, None if mode == "fp8dr" else wtt_sb, ncols_s)
                            mm_into(ps_h[:, j * 512:j * 512 + ncols_s], o, xtiles, xdr,
                                    w3h, None if mode == "fp8dr" else wht_sb, ncols_s)
                        tau = work.tile([128, 1024], BF16, tag="taum", bufs=4, name="tau")
                        nc.scalar.activation(tau[:], ps_t[:], AF.Sigmoid,
                                             bias=cbt[:], scale=1.0)
                        hr = work.tile([128, 1024], BF16, tag="hrm", bufs=4, name="hr")
                        nc.scalar.activation(hr[:], ps_h[:], AF.Relu,
                                             bias=cbh[:], scale=1.0)
                        for j in range(2):
                            o = 2 * og + j
                            for r in range(nrows_here):
                                sl = slice(j * 512 + r * S, j * 512 + (r + 1) * S)
                                slx = slice(r * S, (r + 1) * S)
                                col = col0 + r
                                s1 = work.tile([128, S], BF16, tag="scr1", bufs=3, name="s1")
                                s2 = work.tile([128, S], BF16, tag="scr2", bufs=3, name="s2")
                                nc.vector.scalar_tensor_tensor(
                                    out=s1[:], in0=tau[:, sl], scalar=1.0, in1=hr[:, sl],
                                    op0=ALU.mult, op1=ALU.mult,
                                    accum_out=acc_a[o][:, col:col + 1])
                                eng2 = nc.gpsimd if stt_split else nc.vector
                                eng2.scalar_tensor_tensor(
                                    out=s2[:], in0=tau[:, sl], scalar=1.0,
                                    in1=xtiles[o][:, slx],
                                    op0=ALU.subtract, op1=ALU.mult,
                                    accum_out=i_sb[o][:, col:col + 1])

                # per "pair": 2 K-rows (or the single Q row for pair index `pairs`)
                def do_pair(ps1, xtiles, ncols_s, col0, nrows_here, xdr=None):
                    # xtiles: EC tiles [128, nrows_here*S] bf16  (X.T chunks)
                    # xdr (fp8dr): EC//2 tiles, 3D [128, 2, nrows_here*S] fp8
                    if const_bias is not None:
                        do_pair_merged(ps1, xtiles, ncols_s, col0, nrows_here, xdr)
                        return
                    for o in range(EC):
                        ps_t = ps1.tile([128, 512], F32, tag="psT", bufs=4)
                        ps_h = ps1.tile([128, 512], F32, tag="psH", bufs=4)
                        if mode == "fp8dr":
                            for c in range(EC // 2):
                                nc.tensor.matmul(
                                    ps_t[:, 0:ncols_s],
                                    wtt3[:, 2 * c:2 * c + 2, o * 128:(o + 1) * 128],
                                    xdr[c][:, :, 0:ncols_s],
                                    start=(c == 0), stop=(c == EC // 2 - 1),
                                    perf_mode=mybir.MatmulPerfMode.DoubleRow)
                            for c in range(EC // 2):
                                nc.tensor.matmul(
                                    ps_h[:, 0:ncols_s],
                                    wht3[:, 2 * c:2 * c + 2, o * 128:(o + 1) * 128],
                                    xdr[c][:, :, 0:ncols_s],
                                    start=(c == 0), stop=(c == EC // 2 - 1),
                                    perf_mode=mybir.MatmulPerfMode.DoubleRow)
                        else:
                            for k in range(EC):
                                nc.tensor.matmul(ps_t[:, 0:ncols_s], wtt_sb[k][:, o * 128:(o + 1) * 128],
                                                 xtiles[k][:, 0:ncols_s], start=(k == 0), stop=(k == EC - 1))
                            for k in range(EC):
                                nc.tensor.matmul(ps_h[:, 0:ncols_s], wht_sb[k][:, o * 128:(o + 1) * 128],
                                                 xtiles[k][:, 0:ncols_s], start=(k == 0), stop=(k == EC - 1))
                        tau = work.tile([128, 512], BF16, tag="tau",
                                        bufs=8 if deep_bufs else 4)
                        nc.scalar.activation(tau[:, 0:ncols_s], ps_t[:, 0:ncols_s], AF.Sigmoid,
                                             bias=bt_sb[o][:], scale=1.0)
                        hr = work.tile([128, 512], BF16, tag="hr",
                                       bufs=8 if deep_bufs else 4)
                        nc.scalar.activation(hr[:, 0:ncols_s], ps_h[:, 0:ncols_s], AF.Relu,
                                             bias=bh_sb[o][:], scale=1.0)
                        for r in range(nrows_here):
                            sl = slice(r * S, (r + 1) * S)
                            col = col0 + r
                            s1 = work.tile([128, S], BF16, tag="scr1",
                                          bufs=6 if deep_bufs else 3)
                            s2 = work.tile([128, S], BF16, tag="scr2",
                                          bufs=6 if deep_bufs else 3)
                            # acc_a = sum_s tau*H'
                            nc.vector.scalar_tensor_tensor(
                                out=s1[:], in0=tau[:, sl], scalar=1.0, in1=hr[:, sl],
                                op0=ALU.mult, op1=ALU.mult,
                                accum_out=acc_a[o][:, col:col + 1])
                            # i_sb = sum_s (tau-1)*X   -> I = acc_a - i_sb
                            eng2 = nc.gpsimd if stt_split else nc.vector
                            eng2.scalar_tensor_tensor(
                                out=s2[:], in0=tau[:, sl], scalar=1.0, in1=xtiles[o][:, sl],
                                op0=ALU.subtract, op1=ALU.mult,
                                accum_out=i_sb[o][:, col:col + 1])

                with tc.tile_pool(name="ps1", bufs=1, space="PSUM") as ps1:
                    # Q first (warms PE, fills col `rows`)
                    xqt = work.tile([128, EC * S], BF16, tag="xq", bufs=1, name="xqt")
                    nc.sync.dma_start(xqt[:], qtp[:, :])
                    xq = [xqt[:, k * S:(k + 1) * S] for k in range(EC)]
                    xqdr = None
                    if mode == "fp8dr":
                        xqd = work.tile([128, EC * S], F8, tag="xqd", bufs=1, name="xqd")
                        nc.sync.dma_start(xqd[:], qtdp[:, :])
                        xqdr = [xqd[:, c * 2 * S:(c + 1) * 2 * S]
                                .rearrange("p (j s) -> p j s", j=2)
                                for c in range(EC // 2)]
                    do_pair(ps1, xq, S, rows, 1, xdr=xqdr)

                    for p in range(pairs):
                        xt = work.tile([128, EC * 2 * S], BF16, tag="xt", bufs=x_bufs,
                                       name="xt")
                        nc.sync.dma_start(xt[:], ktp[p, :, :])
                        xp = [xt[:, k * 2 * S:(k + 1) * 2 * S] for k in range(EC)]
                        xdr = None
                        if mode == "fp8dr":
                            xd = work.tile([128, EC * 2 * S], F8, tag="xd", bufs=x_bufs,
                                           name="xd")
                            dma_eng = nc.gpsimd if dma_split else nc.sync
                            dma_eng.dma_start(xd[:], ktdp[p, :, :])
                            xdr = [xd[:, c * 2 * 2 * S:(c + 1) * 2 * 2 * S]
                                   .rearrange("p (j rs) -> p j rs", j=2)
                                   for c in range(EC // 2)]
                        do_pair(ps1, xp, 2 * S, 2 * p, 2, xdr=xdr)

                # ---- stage 2: critic head + navigator on [128, NCOLS] ---------
                ps2pool = tc.tile_pool(name="ps2", bufs=1, space="PSUM")
                ps2 = ps2pool.__enter__()
                # I = acc_a - i_sb  (sum tau*H' - sum (tau-1)*X), cast to bf16
                i16 = []
                for k in range(EC):
                    t = cst.tile([128, NCOLS], BF16, tag=f"i16{k}")
                    nc.vector.tensor_tensor(out=t[:], in0=acc_a[k][:], in1=i_sb[k][:],
                                            op=ALU.subtract)
                    i16.append(t)

                a16 = []
                for dch in range(DC):
                    ps = ps2.tile([128, NCOLS], F32, tag="ps2", bufs=2, name="ps")
                    for k in range(EC):
                        nc.tensor.matmul(ps[:], wc1_sb[k][:, dch * 128:(dch + 1) * 128],
                                         i16[k][:], start=(k == 0), stop=(k == EC - 1))
                    t = cst.tile([128, NCOLS], BF16, tag=f"a16{dch}")
                    nc.scalar.activation(t[:], ps[:], AF.Lrelu, bias=bc1_sb[dch][:],
                                         scale=1.0, alpha=NEG_SLOPE)
                    a16.append(t)

                f_sb = []
                for fch in range(DC):
                    ps = ps2.tile([128, NCOLS], F32, tag="ps2", bufs=2, name="ps")
                    for k in range(DC):
                        nc.tensor.matmul(ps[:], wc2_sb[k][:, fch * 128:(fch + 1) * 128],
                                         a16[k][:], start=(k == 0), stop=(k == DC - 1))
                    t = cst.tile([128, NCOLS], F32, tag=f"fsb{fch}")
                    nc.scalar.activation(t[:], ps[:], AF.Identity, bias=bc2_sb[fch][:], scale=1.0)
                    f_sb.append(t)

                mse16 = []
                for fch in range(DC):
                    dsub = cst.tile([128, rows], BF16, tag=f"dsub{fch}")
                    nc.vector.tensor_tensor(
                        out=dsub[:], in0=f_sb[fch][:, 0:rows],
                        in1=f_sb[fch][:, rows:rows + 1].to_broadcast((128, rows)),
                        op=ALU.subtract)
                    m = cst.tile([128, rows], BF16, tag=f"mse{fch}")
                    nc.vector.tensor_tensor(out=m[:], in0=dsub[:], in1=dsub[:], op=ALU.mult)
                    mse16.append(m)

                ps_c = ps2.tile([1, rows], F32, tag="psc", bufs=1)
                for k in range(DC):
                    nc.tensor.matmul(ps_c[:], ones16[:, 0:1], mse16[k][:],
                                     start=(k == 0), stop=(k == DC - 1))
                c_sb = cst.tile([1, rows], F32, tag="csb")
                nc.vector.tensor_copy(c_sb[:], ps_c[:])
                nc.sync.dma_start(c_out[:, :], c_sb[:])

                h1 = []
                for hch in range(HC):
                    ps = ps2.tile([128, rows], F32, tag="ps2", bufs=2, name="ps")
                    for k in range(DC):
                        nc.tensor.matmul(ps[:], wn1_sb[k][:, hch * 128:(hch + 1) * 128],
                                         mse16[k][:], start=(k == 0), stop=(k == DC - 1))
                    t = cst.tile([128, rows], BF16, tag=f"h1_{hch}")
                    nc.scalar.activation(t[:], ps[:], AF.Lrelu, bias=bn1_sb[hch][:],
                                         scale=1.0, alpha=NEG_SLOPE)
                    h1.append(t)

                h2 = []
                for gch in range(DC):
                    ps = ps2.tile([128, rows], F32, tag="ps2", bufs=2, name="ps")
                    for k in range(HC):
                        nc.tensor.matmul(ps[:], wn2_sb[k][:, gch * 128:(gch + 1) * 128],
                                         h1[k][:], start=(k == 0), stop=(k == HC - 1))
                    t = cst.tile([128, rows], BF16, tag=f"h2_{gch}")
                    nc.scalar.activation(t[:], ps[:], AF.Lrelu, bias=bn2_sb[gch][:],
                                         scale=1.0, alpha=NEG_SLOPE)
                    h2.append(t)

                ps_d = ps2.tile([1, rows], F32, tag="psd", bufs=1)
                for k in range(DC):
                    nc.tensor.matmul(ps_d[:], wn3_sb[k][:, 0:1], h2[k][:],
                                     start=(k == 0), stop=(k == DC - 1))
                d_sb = cst.tile([1, rows], F32, tag="dsb")
                nc.scalar.activation(d_sb[:], ps_d[:], AF.Identity,
                                     bias=nbn3_sb[0:1, :], scale=-1.0)
                nc.sync.dma_start(d_out[:, :], d_sb[:])
                ps2pool.__exit__(None, None, None)

    nc.compile()
    return nc


_CACHED = {}
MODE = "fp8dr"
STT_SPLIT = False


def _program(rows=ROWS, const_bias=None):
    key = (rows, MODE, STT_SPLIT, const_bias)
    if key not in _CACHED:
        _CACHED[key] = _build(rows, mode=MODE, stt_split=STT_SPLIT,
                              const_bias=const_bias, dma_split=True)
    return _CACHED[key]


def _dr_pack(wt_t):
    # [E_contract, ncol] -> DR-interleaved [128, EC*ncol] fp8:
    # out[p, j*ncol + o] = wt_t[128*j + p, o]
    e, ncol = wt_t.shape
    j = e // 128
    return np.ascontiguousarray(
        wt_t.reshape(j, 128, ncol).transpose(1, 0, 2).reshape(128, j * ncol)
    ).astype(NP8)


def kernel(K, Q, WT, bT, WH, bH, Wc1, bc1, Wc2, bc2, Wn1, bn1, Wn2, bn2, Wn3, bn3):
    K = np.asarray(K)
    Q = np.asarray(Q)
    bT = np.asarray(bT, np.float32)
    bH = np.asarray(bH, np.float32)
    # NOTE: the merged (FD=1024) const-bias ACT path exists (const_bias=...)
    # but measured ~9% SLOWER on hardware than the per-chunk path (the 2-bank
    # PSUM tiles at bufs=2 cost more matmul/ACT overlap than the merged
    # activations save), so the per-chunk path is the shipped default.
    nc = _program(const_bias=None)

    q_t = np.ascontiguousarray(np.asarray(Q).T)  # [E, S]
    common = {
        "QTP": np.ascontiguousarray(
            q_t.reshape(EC, 128, S).transpose(1, 0, 2).reshape(128, EC * S)
        ).astype(BF),
        "WC1T": np.ascontiguousarray(np.asarray(Wc1).T).astype(BF),
        "WC2T": np.ascontiguousarray(np.asarray(Wc2).T).astype(BF),
        "WN1T": np.ascontiguousarray(np.asarray(Wn1).T).astype(BF),
        "WN2T": np.ascontiguousarray(np.asarray(Wn2).T).astype(BF),
        "WN3T": np.ascontiguousarray(np.asarray(Wn3).T).astype(BF),
        "BT": np.asarray(bT, np.float32).reshape(E, 1),
        "BH": np.asarray(bH, np.float32).reshape(E, 1),
        "BC1": np.asarray(bc1, np.float32).reshape(DIM, 1),
        "BC2": np.asarray(bc2, np.float32).reshape(DIM, 1),
        "BN1": np.asarray(bn1, np.float32).reshape(HID, 1),
        "BN2": np.asarray(bn2, np.float32).reshape(DIM, 1),
        "BN3": np.asarray(bn3, np.float32).reshape(1, 1),
    }
    wt_t = np.ascontiguousarray(np.asarray(WT).T)
    wh_t = np.ascontiguousarray(np.asarray(WH).T)
    if MODE == "fp8dr":
        common["WTTD"] = _dr_pack(wt_t)
        common["WHTD"] = _dr_pack(wh_t)
        common["QTDP"] = np.ascontiguousarray(
            q_t.reshape(EC // 2, 2, 128, S).transpose(2, 0, 1, 3)
            .reshape(128, EC * S)).astype(NP8)
    else:
        common["WTT"] = wt_t.astype(BF)
        common["WHT"] = wh_t.astype(BF)
    # K [N, S, E] -> X.T [N, E, S], then pack per PAIR of rows:
    #   bf16: [pairs, 128, (k, r, s)]   fp8 DR: [pairs, 128, (c, j, r, s)]
    kt = np.ascontiguousarray(K.transpose(0, 2, 1))  # [N, E, S]
    npair = N // 2
    ktp = np.ascontiguousarray(
        kt.reshape(npair, 2, EC, 128, S).transpose(0, 3, 2, 1, 4)
        .reshape(npair, 128, EC * 2 * S)).astype(BF)
    in_maps = [dict(common, KTP=ktp[c * ROWS // 2:(c + 1) * ROWS // 2])
               for c in range(N_CORES)]
    if MODE == "fp8dr":
        ktdp = np.ascontiguousarray(
            kt.reshape(npair, 2, EC // 2, 2, 128, S).transpose(0, 4, 2, 3, 1, 5)
            .reshape(npair, 128, EC * 2 * S)).astype(NP8)
        for c in range(N_CORES):
            in_maps[c]["KTDP"] = ktdp[c * ROWS // 2:(c + 1) * ROWS // 2]
    global _last_in_maps
    _last_in_maps = in_maps

    res = run_bass_kernel_spmd(nc, in_maps, list(range(N_CORES))).results

    c = np.concatenate([res[i]["C_OUT"][0] for i in range(N_CORES)]).astype(np.float32)
    d = np.concatenate([res[i]["D_OUT"][0] for i in range(N_CORES)]).astype(np.float32)
    e = np.exp(d - d.max(), dtype=np.float32)
    sm = e / e.sum(dtype=np.float32)
    loss = RHO * c.mean(dtype=np.float32) + (1.0 - RHO) * np.sum(c * sm, dtype=np.float32)
    return np.asarray(loss, dtype=np.float32)

